# revision 31
# baseline (speedup 1.0000x reference)
"""Distributional (Gaussian-KL) attention on 8 TRN2 NeuronCores.

Math: for each head, the KL-based score decomposes as
    kl[q,k] = sum_d (Qm-Km)^2/(2Kv) + 0.5*(Qv/Kv - log(Qv/Kv) - 1)
            = Fq[q] . Fk[k] + r[k] + c[q]
with  Fq = [Qm^2+Qv ; -Qm],  Fk = [1/(2Kv) ; Km/Kv],
      r  = 0.5*sum_d (Km^2/Kv + log Kv),
and c[q] only shifts softmax logits per-row (drops out of softmax over k).
scores = -kl/sqrt(Dh); kl >= 0 so exp(scores) needs no max-shift.

Sharding: head-parallel. Core c owns heads {2c, 2c+1} == feature columns
[128c, 128c+128) of every Q/K/V projection. The host supplies X already
TRANSPOSED ([D, L]) and everything pre-cast to bf16, so the device does
no input casts and no X transposes. V is projected sequence-major
(lhsT = X^T tiles) so PV needs no V transpose either.

All transcendentals use ONE ACT table set (natural_log_exp_and_others):
softplus(x) = ln(exp(x) + 1)  (Exp then Ln with bias=1.0), and
1/Kv = exp(-ln Kv) reuses the ln Kv needed for the r term. No sigmoid.

Communication: ONE merged AllGather (o_var | o_mu packed into a
[128, 512] bf16 buffer). In this axon-relay environment the collective
stream's first service lands ~65-80us after kernel start regardless of
trigger time, and completions of split collectives are batched (so a
var/mu split yields NO overlap, just an extra handoff and a worse data
rate). Post-gather, the [1024, 512] result is read back over three DMA
queues (sync/scalar/gpsimd; Shared-DRAM reads run ~67GB/s per queue)
feeding both output projections concurrently.
The r_k bias path stays true fp32 (r is O(300); bf16 rounding there
shifts logits by ~0.15) and enters PSUM as a hi+lo bf16 split.
"""

import numpy as np
import ml_dtypes

import concourse.bass as bass
import concourse.mybir as mybir
import concourse.tile as tile
from concourse import bacc
from concourse.masks import make_identity
from concourse.bass_utils import run_bass_kernel_spmd

F32 = mybir.dt.float32
BF16 = mybir.dt.bfloat16
AF = mybir.ActivationFunctionType
ALU = mybir.AluOpType

H, B, L, D = 16, 1, 256, 1024
Dh = D // H          # 64
NCORES = 8
CB = D // NCORES     # 128 feature columns per core (2 heads)
P = 128
LT = L // P          # 2 row tiles of the sequence
KT = D // P          # 8 contraction tiles

TRACE = False
TRACE_KWARGS = {}
LAST_RESULT = None

_prog_cache = {}


def ts(i, size):
    return slice(i * size, (i + 1) * size)


def build_program():
    # The table-load pass picks the FIRST act-function set containing a
    # requested function (exp -> exp_and_others, ln -> natural_log), so a
    # kernel alternating Exp/Ln reloads tables on every switch (~1.5us
    # each). Everything this kernel needs (exp, ln, square, copy) lives in
    # natural_log_exp_and_others; restrict the candidate list during OUR
    # compile so exactly one load is emitted, then restore.
    orig_get_tables = bacc.get_activation_tables

    def only_ln_exp(arch):
        # Keep list length/order (set id = index into act_info.json);
        # empty every other set so first-match lands on the ln+exp set.
        full = orig_get_tables(arch)
        keep = "natural_log_exp_and_others"
        return {name: (fns if name == keep else set())
                for name, fns in full.items()}

    nc = bacc.Bacc("TRN2", target_bir_lowering=False, debug=False,
                   num_devices=NCORES)

    xt_mu_d = nc.dram_tensor("xt_mu", [D, L], BF16, kind="ExternalInput")
    xt_var_d = nc.dram_tensor("xt_var", [D, L], BF16, kind="ExternalInput")
    w_names = ["wq_var", "wk_var", "wq_mu", "wk_mu", "wv_var", "wv_mu"]
    w_d = {n: nc.dram_tensor(n, [D, CB], BF16, kind="ExternalInput")
           for n in w_names}
    wo_mu_d = nc.dram_tensor("wo_mu", [D, CB], BF16, kind="ExternalInput")
    wo_var_d = nc.dram_tensor("wo_var", [D, CB], BF16, kind="ExternalInput")
    b_d = nc.dram_tensor("biases", [CB, 8], F32, kind="ExternalInput")
    bv_d = nc.dram_tensor("bvrow", [2, CB], BF16, kind="ExternalInput")
    out_mu_d = nc.dram_tensor("out_mu", [CB, L], F32, kind="ExternalOutput")
    out_var_d = nc.dram_tensor("out_var", [CB, L], F32, kind="ExternalOutput")

    with tile.TileContext(nc) as tc:
        _build(nc, tc, xt_mu_d, xt_var_d, w_d, wo_mu_d, wo_var_d, b_d, bv_d,
               out_mu_d, out_var_d)
    bacc.get_activation_tables = only_ln_exp
    try:
        nc.compile()
    finally:
        bacc.get_activation_tables = orig_get_tables
    return nc


def _build(nc, tc, xt_mu_d, xt_var_d, w_d, wo_mu_d, wo_var_d, b_d, bv_d,
           out_mu_d, out_var_d):
    from contextlib import ExitStack
    ctx = ExitStack()
    with ctx:
        const = ctx.enter_context(tc.tile_pool(name="const", bufs=1))
        wpool = ctx.enter_context(tc.tile_pool(name="wpool", bufs=1))
        stage = ctx.enter_context(tc.tile_pool(name="stage", bufs=1))
        # PSUM: 8 banks of 2KB/partition; every buffer rounds up to a
        # bank. One shared [128, 256] f32 pool serves all matmul groups
        # (projections, V, scores, PV, out-proj); transposes + warmups
        # share ps_tr; r gets its own bank. 4 + 3 + 1 = 8 banks.
        ps_mm = ctx.enter_context(
            tc.tile_pool(name="ps_mm", bufs=4, space="PSUM"))
        dram = ctx.enter_context(tc.tile_pool(name="dram", bufs=1,
                                              space="DRAM"))
        # Attention-phase pools: entered LAST (top of the pool stack) and
        # closed right after the collective trigger, so their drains run
        # during the collective wait window, not the final teardown.
        inner = ExitStack()
        feat = inner.enter_context(tc.tile_pool(name="feat", bufs=1))
        attnp = inner.enter_context(tc.tile_pool(name="attnp", bufs=3))
        ps_tr = inner.enter_context(
            tc.tile_pool(name="ps_tr", bufs=3, space="PSUM"))
        ps_r = inner.enter_context(
            tc.tile_pool(name="ps_r", bufs=1, space="PSUM"))

        # ---------------- constants -----------------------------------
        ident_b = const.tile([P, P], BF16, tag="ident_b", name="ident_b")
        make_identity(nc, ident_b)
        ones2 = const.tile([2, P], BF16, tag="ones2", name="ones2")
        nc.vector.memset(ones2, 1.0)
        # halfsel: col h = 0.5 on rows [64h, 64h+64) -- one fp32 matmul
        # gives both heads' r rows at once.
        halfsel = const.tile([P, 2], F32, tag="halfsel", name="halfsel")
        nc.gpsimd.memset(halfsel, 0.0)
        nc.gpsimd.memset(halfsel[0:Dh, 0:1], 0.5)
        nc.gpsimd.memset(halfsel[Dh:P, 1:2], 0.5)

        # ACT warm: trigger the (single) table load at t~0.
        warm_src = const.tile([1, 2], F32, tag="warm_src", name="warm_src")
        nc.vector.memset(warm_src, 0.0)
        warm_dst = const.tile([1, 2], F32, tag="warm_dst", name="warm_dst")
        nc.scalar.activation(warm_dst, warm_src, AF.Exp)
        nc.scalar.activation(warm_dst, warm_src, AF.Ln, bias=1.0)


        # ---------------- input DMAs -----------------------------------
        # var-side on the sync queue, mu-side + output weights on the
        # gpsimd queue (after its tiny const work). X transfers are split
        # into 2-ktile chunks so the first projection matmul can start as
        # soon as its first contraction tile lands.
        xt = {}
        w_bf = {}

        def load_w(dram_t, name, eng):
            wb = wpool.tile([P, KT, CB], BF16, tag=f"w_{name}",
                            name=f"w_{name}")
            eng.dma_start(
                wb, dram_t.ap().rearrange("(kt p) m -> p kt m", p=P))
            return wb

        def load_x(dram_t, name, eng):
            xb = wpool.tile([P, KT, L], BF16, tag=name, name=name)
            src = dram_t.ap().rearrange("(kt p) l -> p kt l", p=P)
            for c in range(0, KT, 2):
                eng.dma_start(xb[:, c:c + 2, :], src[:, c:c + 2, :])
            return xb

        w_bf["wk_var"] = load_w(w_d["wk_var"], "wk_var", nc.sync)
        xt["var"] = load_x(xt_var_d, "xt_var", nc.sync)
        w_bf["wq_var"] = load_w(w_d["wq_var"], "wq_var", nc.sync)
        w_bf["wv_var"] = load_w(w_d["wv_var"], "wv_var", nc.scalar)
        bcat = const.tile([CB, 8], F32, tag="bcat", name="bcat")
        nc.sync.dma_start(bcat, b_d.ap())
        # separate [1, CB] tiles: PE rhs operands must sit at partition 0
        bvrow_mu = const.tile([1, CB], BF16, tag="bvrow_mu", name="bvrow_mu")
        nc.sync.dma_start(bvrow_mu, bv_d.ap()[0:1, :])
        bvrow_var = const.tile([1, CB], BF16, tag="bvrow_var",
                               name="bvrow_var")
        nc.sync.dma_start(bvrow_var, bv_d.ap()[1:2, :])

        w_bf["wk_mu"] = load_w(w_d["wk_mu"], "wk_mu", nc.gpsimd)
        xt["mu"] = load_x(xt_mu_d, "xt_mu", nc.gpsimd)
        w_bf["wq_mu"] = load_w(w_d["wq_mu"], "wq_mu", nc.gpsimd)
        w_bf["wv_mu"] = load_w(w_d["wv_mu"], "wv_mu", nc.gpsimd)
        wo_var_bf = load_w(wo_var_d, "wo_var", nc.scalar)
        wo_mu_bf = load_w(wo_mu_d, "wo_mu", nc.scalar)

        B_ORDER = ["bq_mu", "bq_var", "bk_mu", "bk_var", "bo_mu", "bo_var"]
        nbcat = const.tile([CB, 8], F32, tag="nbcat", name="nbcat")
        nc.scalar.mul(nbcat, bcat, -1.0)
        bias = {n: bcat[:, i:i + 1] for i, n in enumerate(B_ORDER)}
        nbias = {n: nbcat[:, i:i + 1] for i, n in enumerate(B_ORDER)}

        # ---------------- PE warmup (runs during the DMA phase) --------
        for _w in range(8):
            wmp = ps_tr.tile([P, P], F32, tag="tr", name="warm")
            nc.tensor.matmul(wmp, ident_b, ident_b, start=True, stop=True)

        # ---------------- Q/K projections (feature-major [CB, L]) ------
        def project(wb, x_tiles):
            ps = ps_mm.tile([P, L], F32, tag="mm", name="proj")
            for kt in range(KT):
                nc.tensor.matmul(ps, wb[:, kt, :], x_tiles[:, kt, :],
                                 start=(kt == 0), stop=(kt == KT - 1))
            return ps

        # k-var first: its ACT chain (exp->ln->ln->exp) is the deepest.
        ps_kv = project(w_bf["wk_var"], xt["var"])
        ek = feat.tile([P, L], F32, tag="ek", name="ek")
        nc.scalar.activation(ek, ps_kv, AF.Exp, bias=bias["bk_var"])
        t_kv = feat.tile([P, L], F32, tag="t_kv", name="t_kv")
        nc.scalar.activation(t_kv, ek, AF.Ln, bias=1.0)       # Kv
        t_lg = feat.tile([P, L], F32, tag="t_lg", name="t_lg")
        nc.scalar.activation(t_lg, t_kv, AF.Ln)               # ln Kv
        t_iv = feat.tile([P, L], F32, tag="t_iv", name="t_iv")
        nc.scalar.activation(t_iv, t_lg, AF.Exp, scale=-1.0)  # 1/Kv

        ps_qv = project(w_bf["wq_var"], xt["var"])
        eq = feat.tile([P, L], F32, tag="eq", name="eq")
        nc.scalar.activation(eq, ps_qv, AF.Exp, bias=bias["bq_var"])
        t_qv = feat.tile([P, L], F32, tag="t_qv", name="t_qv")
        nc.scalar.activation(t_qv, eq, AF.Ln, bias=1.0)       # Qv

        ps_km = project(w_bf["wk_mu"], xt["mu"])
        t_km = feat.tile([P, L], F32, tag="t_km", name="t_km")
        nc.vector.tensor_scalar_add(t_km, ps_km, bias["bk_mu"])

        ps_qm = project(w_bf["wq_mu"], xt["mu"])
        t_qm2 = feat.tile([P, L], F32, tag="t_qm2", name="t_qm2")
        nc.scalar.activation(t_qm2, ps_qm, AF.Square, bias=bias["bq_mu"])
        negqm_bf = feat.tile([P, L], BF16, tag="negqm", name="negqm")
        nc.vector.tensor_scalar(negqm_bf, ps_qm, -1.0, nbias["bq_mu"],
                                ALU.mult, ALU.add)

        # ---------------- V projections (sequence-major [L, CB]) -------
        # lhsT = X^T tile, rhs = W tile; the per-dout bias enters PSUM via
        # a K=1 ones-row matmul (bias lives on the free axis here).
        v_sb = {}
        for nm, wkey, xkey, brow in (("vv", "wv_var", "var", bvrow_var),
                                     ("vm", "wv_mu", "mu", bvrow_mu)):
            psv = ps_mm.tile([P, L], F32, tag="mm", name=f"vps_{nm}")
            for t in range(LT):
                for kt in range(KT):
                    nc.tensor.matmul(psv[:, ts(t, P)],
                                     xt[xkey][:, kt, ts(t, P)],
                                     w_bf[wkey][:, kt, :],
                                     start=(kt == 0), stop=False)
                nc.tensor.matmul(psv[:, ts(t, P)], ones2[0:1, :], brow,
                                 start=False, stop=True)
            dst = feat.tile([P, L], BF16, tag=f"v_{nm}", name=f"v_{nm}")
            if nm == "vv":
                ev = stage.tile([P, L], F32, tag="ev", name="ev")
                nc.scalar.activation(ev, psv, AF.Exp)
                nc.scalar.activation(dst, ev, AF.Ln, bias=1.0)
            else:
                nc.vector.tensor_copy(dst, psv)
            v_sb[nm] = dst

        # ---------------- derived features (DVE) ------------------------
        t_km2 = feat.tile([P, L], F32, tag="t_km2", name="t_km2")
        nc.vector.tensor_mul(t_km2, t_km, t_km)
        fk1_bf = feat.tile([P, L], BF16, tag="fk1", name="fk1")
        nc.vector.tensor_scalar_mul(fk1_bf, t_iv, 0.5)
        t_kiv = feat.tile([P, L], F32, tag="t_kiv", name="t_kiv")
        nc.vector.tensor_mul(t_kiv, t_km, t_iv)
        kmiv_bf = feat.tile([P, L], BF16, tag="kmiv", name="kmiv")
        nc.vector.tensor_copy(kmiv_bf, t_kiv)
        t_u = feat.tile([P, L], F32, tag="t_u", name="t_u")
        nc.vector.tensor_mul(t_u, t_km2, t_iv)
        t_s = feat.tile([P, L], F32, tag="t_s", name="t_s")
        nc.vector.tensor_add(t_s, t_u, t_lg)
        fq1_bf = feat.tile([P, L], BF16, tag="fq1", name="fq1")
        nc.vector.tensor_add(fq1_bf, t_qm2, t_qv)

        # r row per head (fp32 matmul, M=1), then a hi+lo bf16 split so
        # the O(300) values enter the score PSUM without bf16 rounding.
        # Engine APs must start at a 32-aligned partition, so each head
        # gets its own partition-0 [1, L] tiles.
        r_hi, r_lo = [], []
        for h in range(2):
            pr = ps_r.tile([1, L], F32, tag="r_ps", name=f"r_ps_{h}")
            nc.tensor.matmul(pr, halfsel[:, h:h + 1], t_s,
                             start=True, stop=True)
            hi = feat.tile([1, L], BF16, tag=f"r_hi_{h}", name=f"r_hi_{h}")
            nc.vector.tensor_copy(hi, pr)
            lof = feat.tile([1, L], F32, tag=f"r_lof_{h}",
                            name=f"r_lof_{h}")
            nc.vector.tensor_sub(lof, pr, hi)
            lo = feat.tile([1, L], BF16, tag=f"r_lo_{h}", name=f"r_lo_{h}")
            nc.vector.tensor_copy(lo, lof)
            r_hi.append(hi)
            r_lo.append(lo)

        # ---------------- attention ------------------------------------
        # kl >= 0 and max_k kl/8 is O(10) => exp without max-subtraction
        attnT = {}   # (h, lk) -> [128 (k within lk), 256 (q)] bf16
        a2T = {}
        for h in range(2):
            hs = ts(h, Dh)
            for t in range(LT):
                ps_S = ps_mm.tile([P, L], F32, tag="mm", name="scores")
                nc.tensor.matmul(ps_S, fq1_bf[hs, ts(t, P)], fk1_bf[hs, :],
                                 start=True, stop=False)
                nc.tensor.matmul(ps_S, negqm_bf[hs, ts(t, P)], kmiv_bf[hs, :],
                                 start=False, stop=False)
                nc.tensor.matmul(ps_S, ones2[0:1, :], r_hi[h],
                                 start=False, stop=False)
                nc.tensor.matmul(ps_S, ones2[0:1, :], r_lo[h],
                                 start=False, stop=True)
                pexp = attnp.tile([P, L], BF16, tag="pexp", name="pexp")
                den = attnp.tile([P, 1], F32, tag="den", name="den")
                nc.scalar.activation(pexp, ps_S, AF.Exp, bias=0.0,
                                     scale=-0.125, accum_out=den)
                invd = attnp.tile([P, 1], F32, tag="invd", name="invd")
                nc.vector.reciprocal(invd, den)
                a_bf = attnp.tile([P, L], BF16, tag="a_bf", name="a_bf")
                nc.vector.tensor_scalar_mul(a_bf, pexp, invd)
                for lk in range(LT):
                    if (h, lk) not in attnT:
                        attnT[(h, lk)] = feat.tile(
                            [P, L], BF16, tag=f"attnT_{h}_{lk}",
                            name=f"attnT_{h}_{lk}")
                    pt = ps_tr.tile([P, P], BF16, tag="tr", name="trb")
                    nc.tensor.transpose(pt, a_bf[:, ts(lk, P)], ident_b)
                    # GPSIMD has no PSUM port; split the copies between
                    # DVE and ACT so neither engine serializes them all.
                    if lk == 0:
                        nc.vector.tensor_copy(attnT[(h, lk)][:, ts(t, P)], pt)
                    else:
                        nc.scalar.copy(attnT[(h, lk)][:, ts(t, P)], pt)
        for h in range(2):
            for lk in range(LT):
                a2 = feat.tile([P, L], BF16, tag=f"a2_{h}_{lk}",
                               name=f"a2_{h}_{lk}")
                nc.scalar.activation(a2, attnT[(h, lk)], AF.Square)
                a2T[(h, lk)] = a2

        # ---------------- PV + single AllGather -------------------------
        # The relay batches collective completions, so a split var/mu
        # gather gives NO overlap (observed: all post-gather work waits
        # for the LAST collective anyway) while paying an extra handoff
        # and a worse data rate. One merged gather: o_var in cols [0,L),
        # o_mu in cols [L,2L).
        cc_in = dram.tile([CB, 2 * L], BF16, tag="cc_in", name="cc_in")
        pv_var = ps_mm.tile([P, L], F32, tag="mm", name="pv_var")
        for h in range(2):
            for lk in range(LT):
                nc.tensor.matmul(pv_var[ts(h, Dh), :],
                                 v_sb["vv"][:, lk * P + h * Dh:
                                            lk * P + (h + 1) * Dh],
                                 a2T[(h, lk)],
                                 start=(lk == 0), stop=(lk == LT - 1),
                                 tile_position=(0, h * Dh))
        o_var = attnp.tile([P, L], BF16, tag="o_var", name="o_var")
        nc.vector.tensor_copy(o_var, pv_var)
        nc.sync.dma_start(cc_in[:, 0:L], o_var)

        pv_mu = ps_mm.tile([P, L], F32, tag="mm", name="pv_mu")
        for h in range(2):
            for lk in range(LT):
                nc.tensor.matmul(pv_mu[ts(h, Dh), :],
                                 v_sb["vm"][:, lk * P + h * Dh:
                                            lk * P + (h + 1) * Dh],
                                 attnT[(h, lk)],
                                 start=(lk == 0), stop=(lk == LT - 1),
                                 tile_position=(0, h * Dh))
        o_mu = attnp.tile([P, L], BF16, tag="o_mu", name="o_mu")
        nc.vector.tensor_copy(o_mu, pv_mu)
        nc.sync.dma_start(cc_in[:, L:2 * L], o_mu)
        cc_out = dram.tile([NCORES * CB, 2 * L], BF16, tag="cc_out",
                           name="cc_out", addr_space="Shared")
        nc.gpsimd.collective_compute(
            "AllGather", ALU.bypass,
            replica_groups=[list(range(NCORES))],
            ins=[cc_in[:].opt()],
            outs=[cc_out[:].opt()],
        )
        inner.close()

        # ---------------- output projections ---------------------------
        # Per-chunk DMAs from the gathered buffer so the first matmul
        # starts as soon as possible after the collective completes.
        def out_proj(col0, wo_bf_t, qname, engs):
            # 2-ktile chunks spread over the given queues (Shared-DRAM
            # reads run ~67GB/s per queue; ~2us per 128KB chunk).
            g = []
            for ci, c in enumerate(range(0, KT, 2)):
                gt = stage.tile([P, 2, L], BF16, tag=f"g_{qname}_{c}",
                                name=f"g_{qname}_{c}")
                engs[ci % len(engs)].dma_start(
                    gt, cc_out[c * P:(c + 2) * P, col0:col0 + L]
                    .rearrange("(c p) l -> p c l", p=P))
                g.append(gt)
            ps = ps_mm.tile([P, L], F32, tag="mm", name="oproj")
            for kt in range(KT):
                nc.tensor.matmul(ps, wo_bf_t[:, kt, :], g[kt // 2][:, kt % 2, :],
                                 start=(kt == 0), stop=(kt == KT - 1))
            return ps

        ps_ovar = out_proj(0, wo_var_bf, "var",
                           [nc.sync, nc.gpsimd, nc.sync, nc.gpsimd])
        eo = stage.tile([P, L], F32, tag="eo", name="eo")
        nc.scalar.activation(eo, ps_ovar, AF.Exp, bias=bias["bo_var"])
        res_var = stage.tile([P, L], F32, tag="res_var", name="res_var")
        nc.scalar.activation(res_var, eo, AF.Ln, bias=1.0)
        nc.sync.dma_start(out_var_d.ap(), res_var)

        ps_omu = out_proj(L, wo_mu_bf, "mu",
                          [nc.scalar, nc.sync, nc.scalar, nc.gpsimd])
        res_mu = stage.tile([P, L], F32, tag="res_mu", name="res_mu")
        nc.vector.tensor_scalar_add(res_mu, ps_omu, bias["bo_mu"])
        nc.scalar.dma_start(out_mu_d.ap(), res_mu)


def shard_inputs(inputs):
    """Full inputs -> per-core in_maps (host-side slicing/layout only)."""
    f32 = np.float32
    bf16 = ml_dtypes.bfloat16
    mu = inputs["mu"].reshape(L, D).astype(f32)
    var = inputs["var"].reshape(L, D).astype(f32)
    xt_mu = np.ascontiguousarray(mu.T).astype(bf16)
    xt_var = np.ascontiguousarray(var.T).astype(bf16)
    in_maps = []
    for c in range(NCORES):
        cols = slice(c * CB, (c + 1) * CB)
        m = {"xt_mu": xt_mu, "xt_var": xt_var}
        for n in ["wq_mu", "wq_var", "wk_mu", "wk_var", "wv_mu", "wv_var",
                  "wo_mu", "wo_var"]:
            m[n] = np.ascontiguousarray(
                inputs[n][:, cols].astype(f32)).astype(bf16)
        bnames = ["bq_mu", "bq_var", "bk_mu", "bk_var", "bo_mu", "bo_var"]
        bb = np.zeros((CB, 8), f32)
        for i, n in enumerate(bnames):
            bb[:, i] = inputs[n][cols].astype(f32)
        m["biases"] = bb
        m["bvrow"] = np.ascontiguousarray(np.stack(
            [inputs["bv_mu"][cols].astype(f32),
             inputs["bv_var"][cols].astype(f32)], axis=0)).astype(bf16)
        in_maps.append(m)
    return in_maps


def kernel(**inputs):
    global LAST_RESULT
    if "prog" not in _prog_cache:
        _prog_cache["prog"] = build_program()
    nc = _prog_cache["prog"]
    in_maps = shard_inputs(inputs)
    res = run_bass_kernel_spmd(nc, in_maps, core_ids=list(range(NCORES)),
                               trace=TRACE, **TRACE_KWARGS)
    LAST_RESULT = res
    mu_blocks = [res.results[c]["out_mu"] for c in range(NCORES)]
    var_blocks = [res.results[c]["out_var"] for c in range(NCORES)]
    mu_out = np.concatenate(mu_blocks, axis=0).T.reshape(B, L, D)
    var_out = np.concatenate(var_blocks, axis=0).T.reshape(B, L, D)
    return (np.ascontiguousarray(mu_out.astype(np.float32)),
            np.ascontiguousarray(var_out.astype(np.float32)))
